# revision 23
# baseline (speedup 1.0000x reference)
"""MoE (top-2 of 8 experts, D=1024, F=4096, T=8192) on 8 TRN2 NeuronCores.

Strategy: expert-parallel. The router (a [T,1024]@[1024,8] matmul + top-2
softmax, ~0.05% of total FLOPs) runs on host with jax-CPU so expert
selection is bit-identical to the reference. Tokens are dispatched to the
core owning their expert (host-side all-to-all as part of sharding), each
core runs the dense FFN relu(x@w1+b1)@w2+b2 over its padded token batch,
and the host combines contributions weighted by the softmax gates.

Device kernel per core: token capacity C = max expert count rounded to 8.
[0, C) is processed in passes; the merged multi-segment pass
(512, C%512, 512) runs FIRST so its long stage-1 window absorbs the w2
resident load, and the kernel ends on a plain 512 pass with a minimal
tail (split drain + one contiguous DMA).

Schedule notes (all trace-verified):
- w2 is fully SBUF-resident (64KB/partition). Its 8 chunk DMAs are
  interleaved into pass-0 stage 1's w1 stream on the SYNC queue --
  serialization there is the only reliable rate limiter (a parallel
  gpsimd bulk load ran at 236GB/s and starved the w1 strips to 78GB/s,
  +20us). All stage-2 phases then run dc-major from residency.
- w1 streams as strip-PAIRS (one DMA per 2 f-chunks) to halve the sync
  queue's per-DMA issue overhead, which starved the baseline's pass-0.
- x and y use per-pass contiguous DRAM layouts [DC, 128, tn]: the
  strided [D, CS] layout made every y DMA a 1KB-per-4.3KB-stride write
  (2.7us for 128KB on the kernel tail).
- pass-0 x chunks split across the gpsimd and scalar queues: descriptor
  generation on one queue serialized ~5us and (on scalar) also blocked
  the stage-1 relu drains, backing up PSUM.
- fp16 compute (fp32 PSUM): the PE streams fp16 at 1 column/cycle, so
  512-col chains at ~216ns are the roofline. fp8 DoubleRow (2x rate)
  measured 5.3e-2 end-to-end rel err -- over the 2e-2 budget -- and
  compensated fp8 needs 3x the columns, so fp16 is optimal.
"""

import numpy as np

D_MODEL = 1024
D_FF = 4096
N_EXPERTS = 8
TOP_K = 2
N_CORES = 8
TILE_N = 512
FC = D_FF // 128   # 32 f-chunks
DC = D_MODEL // 128  # 8 d-chunks

TRACE = False
LAST_EXEC_NS = None
LAST_TRACE_PATH = None

COMPUTE = "fp16"
WARMUP_MMS = 7
CAP_ROUND = 8

_nc_cache = {}


def _r32(v):
    return int(-(-v // 32) * 32)


def _pass_plan(C):
    """Partition [0, C) into passes; each pass is (t0, tn, [seg widths]).

    Segments of one pass share each stage-1 w1 strip. Plain 512 tiles
    plus one merged pass (512, rem, 512); the merged pass is ordered
    FIRST (its long stage 1 hides the w2 resident load) and the kernel
    ends on a plain 512 pass.
    """
    passes = []
    rem = C % TILE_N
    nfull = C // TILE_N
    if nfull >= 2 and rem > 0:
        for i in range(nfull - 2):
            passes.append((i * TILE_N, TILE_N, [TILE_N]))
        t0 = (nfull - 2) * TILE_N
        passes.append((t0, C - t0, [TILE_N, rem, TILE_N]))
    elif rem > 0:
        if nfull > 0:
            for i in range(nfull - 1):
                passes.append((i * TILE_N, TILE_N, [TILE_N]))
            t0 = (nfull - 1) * TILE_N
            passes.append((t0, C - t0, [TILE_N, rem]))
        else:
            passes.append((0, C, [C]))
    else:
        for i in range(max(nfull, 1)):
            passes.append((i * TILE_N, TILE_N, [TILE_N]))
    return passes


def _build_nc(C, CS, compute):
    import concourse.bacc as bacc
    import concourse.tile as tile
    import concourse.mybir as mybir

    f32 = mybir.dt.float32
    cdt = f32 if compute == "fp32" else mybir.dt.float16
    AFT = mybir.ActivationFunctionType

    passes = _pass_plan(C)

    nc = bacc.Bacc("TRN2", target_bir_lowering=False, debug=False,
                   num_devices=N_CORES)
    # per-pass contiguous x/y tensors: [DC, 128, tn] so every chunk DMA
    # is a fully contiguous transfer (the [D, CS] layout made each one
    # a 1-2KB-per-4.3KB-stride access)
    xps, yps = [], []
    for p, (t0, tn, _) in enumerate(passes):
        xps.append(nc.dram_tensor(f"xp{p}", [DC, 128, tn], cdt,
                                  kind="ExternalInput").ap())
        yps.append(nc.dram_tensor(f"yp{p}", [DC, 128, tn], cdt,
                                  kind="ExternalOutput").ap())
    w1p = nc.dram_tensor("w1p", [FC, 128, D_MODEL], cdt,
                         kind="ExternalInput").ap()
    w2p = nc.dram_tensor("w2p", [D_FF, D_MODEL], cdt,
                         kind="ExternalInput").ap()
    b1p = nc.dram_tensor("b1p", [128, FC], f32, kind="ExternalInput").ap()
    b2p = nc.dram_tensor("b2p", [128, DC], f32, kind="ExternalInput").ap()

    w2p_r = w2p.rearrange("(c p) d -> p c d", p=128)  # [128, 32, 1024]
    w1pr = w1p.rearrange("f p d -> p f d")            # [128, 32, 1024]

    with tile.TileContext(nc) as tc:
        with (
            tc.tile_pool(name="const", bufs=1) as constp,
            tc.tile_pool(name="x", bufs=2) as xpool,
            tc.tile_pool(name="h", bufs=1) as hpool,
            tc.tile_pool(name="w1", bufs=4) as w1pool,
            tc.tile_pool(name="w2r", bufs=1) as w2rpool,
            tc.tile_pool(name="o", bufs=2) as opool,
            tc.tile_pool(name="ps", bufs=8, space="PSUM") as pspool,
        ):
            b1s = constp.tile([128, FC], f32)
            nc.scalar.dma_start(b1s[:], b1p)
            b2s = constp.tile([128, DC], f32)
            nc.scalar.dma_start(b2s[:], b2p)

            # warmup fills the PE while the first x/w1 loads land and
            # ramps the activity monitor toward full clock
            warm_w = w1pool.tile([128, 128], cdt, tag="warmw")
            warm_x = xpool.tile([128, 512], cdt, tag="warmx")
            nc.vector.memset(warm_w[:], 0.0)
            nc.vector.memset(warm_x[:], 0.0)
            warm_ps = pspool.tile([128, 512], f32, tag="ps", name="warm_ps")
            for _ in range(WARMUP_MMS):
                nc.tensor.matmul(warm_ps[:], lhsT=warm_w[:], rhs=warm_x[:],
                                 start=True, stop=True)

            def load_xs(pi, split=False):
                tn = passes[pi][1]
                xst = _r32(tn)
                xs = xpool.tile([128, DC * xst], cdt, tag="xs",
                                name=f"xs_{pi}")
                for c in range(DC):
                    if split:
                        eng = nc.gpsimd if c < DC // 2 else nc.scalar
                    else:
                        eng = nc.sync
                    eng.dma_start(xs[:, c * xst:c * xst + tn], xps[pi][c])
                return xs, xst

            def load_w1pair(fc):
                # two consecutive fc strips per DMA: halves the sync
                # queue's per-DMA issue overhead
                w1s = w1pool.tile([128, 2 * D_MODEL], cdt, tag="w1s",
                                  name=f"w1s_{fc}")
                nc.sync.dma_start(w1s[:], w1pr[:, fc:fc + 2, :])
                return w1s

            def load_w1single(fc, eng=None, pool=None):
                # single strip: half the transfer of a pair, so the
                # first f-chunks unblock ~2us earlier at kernel start
                if pool is None:
                    w1s = constp.tile([128, D_MODEL], cdt, tag=f"w1a{fc}",
                                      name=f"w1a_{fc}")
                else:
                    w1s = w1pool.tile([128, D_MODEL], cdt, tag="w1s",
                                      name=f"w1sg_{fc}")
                (eng or nc.sync).dma_start(w1s[:], w1p[fc])
                return w1s

            # startup order on sync: [strip0-half-a, x chunks 0-3 (one
            # DMA), strip0-half-b, x chunks 4-7] -- interleaved at the
            # granularity fc0 consumes them, with batched x to halve
            # descriptor overhead (the sync queue sustains 2-3x the
            # early throughput of gpsimd/scalar: ~200GB/s vs 87
            # measured). strip1 rides the slow scalar queue whose ~14us
            # runway still beats fc1's demand. Only 2 strips hoisted (4
            # flipped the baseline into a +43ns/MM slow mode); fc2/fc3
            # stream as singles, pairs from fc4.
            s0 = constp.tile([128, D_MODEL], cdt, tag="w1a0",
                             name="w1a_0")
            tn0 = passes[0][1]
            xst = _r32(tn0)
            assert xst == tn0
            xs = xpool.tile([128, DC * xst], cdt, tag="xs", name="xs_0")
            xr0 = xps[0].rearrange("c p t -> p c t")
            nc.sync.dma_start(s0[:, :512], w1p[0][:, :512])
            nc.sync.dma_start(xs[:, :4 * xst], xr0[:, 0:4, :])
            nc.sync.dma_start(s0[:, 512:], w1p[0][:, 512:])
            nc.sync.dma_start(xs[:, 4 * xst:], xr0[:, 4:8, :])
            pre0 = {0: s0, 1: load_w1single(1, eng=nc.scalar)}

            # w2 becomes fully SBUF-resident as a side effect of pass-0
            # stage 2: that phase streams each w2 fc-slice just-in-time
            # INTO this tile (fc-outer), so later passes run dc-major
            # from residency with zero extra DMA traffic or contention
            w2r = w2rpool.tile([128, FC * D_MODEL], cdt, name="w2r")

            def stage1(pi, xs, xst, segs, hts, pre=None):
                pair = None
                for fc in range(FC):
                    if pre is not None and fc < 2:
                        pair, woff = pre[fc], 0
                    elif pre is not None and fc in (2, 3):
                        pair, woff = load_w1single(fc, pool=w1pool), 0
                    else:
                        if fc % 2 == 0:
                            pair = load_w1pair(fc)
                        woff = (fc % 2) * D_MODEL
                    pss = [pspool.tile([128, sn], f32, tag="ps",
                                       name=f"ps{pi}_{fc}_{soff}")
                           for soff, sn in segs]
                    for ps, (soff, sn) in zip(pss, segs):
                        for c in range(DC):
                            nc.tensor.matmul(
                                ps[:],
                                lhsT=pair[:, woff + c * 128:
                                          woff + (c + 1) * 128],
                                rhs=xs[:, c * xst + soff:
                                       c * xst + soff + sn],
                                start=(c == 0),
                                stop=(c == DC - 1),
                            )
                    for ps, (soff, sn), (h, hst) in zip(pss, segs, hts):
                        # alternate relu between Scalar and Vector engines
                        # so consecutive psum banks release in parallel
                        if fc % 2 == 0:
                            nc.scalar.activation(
                                h[:, fc * hst:fc * hst + sn], ps[:],
                                AFT.Relu, bias=b1s[:, fc:fc + 1],
                            )
                        else:
                            nc.vector.tensor_scalar(
                                h[:, fc * hst:fc * hst + sn], ps[:],
                                b1s[:, fc:fc + 1], 0.0,
                                mybir.AluOpType.add, mybir.AluOpType.max,
                            )

            def stage2_stream(pi, segs, hts):
                # pass-0 stage 2: fc-outer with 8 dc PSUM banks, each w2
                # fc-slice streamed just-in-time into its final w2r slot.
                # After this phase w2 is fully resident for later passes.
                (soff, tn), (h, hst) = segs[0], hts[0]
                ps2 = [pspool.tile([128, tn], f32, tag="ps",
                                   name=f"p2{pi}_{dc}")
                       for dc in range(DC)]
                for fc in range(FC):
                    nc.sync.dma_start(
                        w2r[:, fc * D_MODEL:(fc + 1) * D_MODEL],
                        w2p_r[:, fc, :])
                    for dc in range(DC):
                        nc.tensor.matmul(
                            ps2[dc][:],
                            lhsT=w2r[:, fc * D_MODEL + dc * 128:
                                     fc * D_MODEL + dc * 128 + 128],
                            rhs=h[:, fc * hst:fc * hst + tn],
                            start=(fc == 0),
                            stop=(fc == FC - 1),
                        )
                for dc in range(DC):
                    outs = opool.tile([128, _r32(tn)], cdt, tag="o0",
                                      name=f"o{pi}_{dc}")
                    if dc % 2 == 0:
                        nc.vector.tensor_scalar_add(
                            outs[:, :tn], ps2[dc][:], b2s[:, dc:dc + 1])
                    else:
                        nc.scalar.activation(
                            outs[:, :tn], ps2[dc][:], AFT.Identity,
                            bias=b2s[:, dc:dc + 1])
                    nc.sync.dma_start(yps[pi][dc], outs[:, :tn])

            def stage2(pi, segs, hts, last=False):
                # dc-major with resident w2: per d-chunk, one 32-MM
                # accumulation chain per segment (widest first), drains
                # as each chain ends, then one contiguous y DMA per dc.
                # On the last pass's final dc, y is split per segment on
                # alternating queues with the narrowest segment last, so
                # the kernel tail is one small drain + one small DMA.
                nseg = len(segs)
                tn = sum(sn for _, sn in segs)
                sorder = sorted(range(nseg), key=lambda s: -segs[s][1])
                eng = 0
                for dc in range(DC):
                    outs = opool.tile([128, _r32(tn)], cdt, tag="o0",
                                      name=f"o{pi}_{dc}")
                    split = last and dc == DC - 1
                    for si, s in enumerate(sorder):
                        soff, sn = segs[s]
                        ps2 = pspool.tile([128, sn], f32, tag="ps",
                                          name=f"p2{pi}_{dc}_{s}")
                        h, hst = hts[s]
                        for fc in range(FC):
                            nc.tensor.matmul(
                                ps2[:],
                                lhsT=w2r[:, fc * D_MODEL + dc * 128:
                                         fc * D_MODEL + dc * 128 + 128],
                                rhs=h[:, fc * hst:fc * hst + sn],
                                start=(fc == 0),
                                stop=(fc == FC - 1),
                            )
                        if split and nseg == 1:
                            # lone segment: drain halves on both engines
                            hn = sn // 2
                            nc.vector.tensor_scalar_add(
                                outs[:, soff:soff + hn], ps2[:, :hn],
                                b2s[:, dc:dc + 1])
                            nc.scalar.activation(
                                outs[:, soff + hn:soff + sn],
                                ps2[:, hn:sn], AFT.Identity,
                                bias=b2s[:, dc:dc + 1])
                        elif eng % 2 == 0:
                            nc.vector.tensor_scalar_add(
                                outs[:, soff:soff + sn], ps2[:],
                                b2s[:, dc:dc + 1])
                        else:
                            nc.scalar.activation(
                                outs[:, soff:soff + sn], ps2[:],
                                AFT.Identity, bias=b2s[:, dc:dc + 1])
                        eng += 1
                        if split and nseg > 1:
                            # per-seg y right after its drain: earlier
                            # segs' transfers overlap later chains
                            q = nc.sync if si % 2 == 0 else nc.scalar
                            q.dma_start(yps[pi][dc][:, soff:soff + sn],
                                        outs[:, soff:soff + sn])
                    if split and nseg == 1:
                        hn = tn // 2
                        nc.sync.dma_start(yps[pi][dc][:, :hn],
                                          outs[:, :hn])
                        nc.scalar.dma_start(yps[pi][dc][:, hn:tn],
                                            outs[:, hn:tn])
                    elif not split:
                        nc.sync.dma_start(yps[pi][dc], outs[:, :tn])

            for pi, (t0, tn, widths) in enumerate(passes):
                segs = []
                off = 0
                for w in widths:
                    segs.append((off, w))
                    off += w
                hts = []
                for s, (soff, sn) in enumerate(segs):
                    hst = _r32(sn)
                    hts.append((hpool.tile([128, FC * hst], cdt,
                                           tag=f"h{s}", name=f"h{pi}_{s}"),
                                hst))
                stage1(pi, xs, xst, segs, hts,
                       pre=pre0 if pi == 0 else None)
                if pi + 1 < len(passes):
                    xs, xst = load_xs(pi + 1)  # prefetch next x on sync
                if pi == 0 and len(passes) > 1:
                    stage2_stream(pi, segs, hts)
                else:
                    stage2(pi, segs, hts, last=(pi == len(passes) - 1))

    nc.compile()
    return nc


def _ensure_trace_hook():
    """bass_utils' axon trace path needs antenv.axon_hooks; inject it."""
    import sys
    import types
    try:
        import antenv
        if "antenv.axon_hooks" in sys.modules:
            return
        from trn_agent_boot.trn_boot import _ntff_profile_via_ctypes
        mod = types.ModuleType("antenv.axon_hooks")
        hook = [_ntff_profile_via_ctypes("/opt/axon/libaxon_pjrt.so")]
        mod.set_axon_ntff_profile_hook = lambda h: hook.__setitem__(0, h)
        mod.get_axon_ntff_profile_hook = lambda: hook[0]
        sys.modules["antenv.axon_hooks"] = mod
        antenv.axon_hooks = mod
    except Exception:
        pass


def _route(xf, router_w, router_b):
    """Top-2 routing, bit-identical to the reference (jax on CPU)."""
    try:
        import jax
        import jax.numpy as jnp

        cpu = jax.devices("cpu")[0]
        with jax.default_device(cpu):
            logits = (jnp.asarray(xf) @ jnp.asarray(router_w)
                      + jnp.asarray(router_b))
            top_vals, top_idx = jax.lax.top_k(logits, TOP_K)
            wts = jax.nn.softmax(top_vals, axis=-1)
        return np.asarray(top_idx), np.asarray(wts, np.float32)
    except Exception:
        # numpy fallback; ties resolve to the lower index like lax.top_k
        logits = xf @ router_w + router_b
        order = np.argsort(-logits, axis=1, kind="stable")[:, :TOP_K]
        vals = np.take_along_axis(logits, order, axis=1)
        ex = np.exp(vals - vals.max(axis=1, keepdims=True))
        wts = (ex / ex.sum(axis=1, keepdims=True)).astype(np.float32)
        return order, wts


def kernel(x, router_w, router_b, w1, b1, w2, b2):
    global LAST_EXEC_NS, LAST_TRACE_PATH
    from concourse import bass_utils

    x = np.asarray(x, np.float32)
    router_w = np.asarray(router_w, np.float32)
    router_b = np.asarray(router_b, np.float32)
    w1 = np.asarray(w1, np.float32)
    b1 = np.asarray(b1, np.float32)
    w2 = np.asarray(w2, np.float32)
    b2 = np.asarray(b2, np.float32)

    orig_shape = x.shape
    xf = x.reshape(-1, x.shape[-1])
    T = xf.shape[0]

    top_idx, wts = _route(xf, router_w, router_b)

    tok_ids = []
    gates = []
    for e in range(N_EXPERTS):
        mask = top_idx == e                      # [T, K]
        sel = mask.any(axis=1)
        ids = np.nonzero(sel)[0]
        # each token picks distinct experts, so at most one k matches
        gk = np.where(mask[ids, 0], wts[ids, 0], wts[ids, 1]).astype(np.float32)
        tok_ids.append(ids)
        gates.append(gk)

    counts = np.array([len(i) for i in tok_ids])
    C = max(512, int(-(-counts.max() // CAP_ROUND) * CAP_ROUND))
    order = np.argsort(counts, kind="stable")  # lightest shard on core 0
    CS = max(512, int(-(-C // 128) * 128))

    key = (C, CS, COMPUTE)
    if key not in _nc_cache:
        _nc_cache[key] = _build_nc(C, CS, COMPUTE)
    nc = _nc_cache[key]

    passes = _pass_plan(C)
    cnp = np.float32 if COMPUTE == "fp32" else np.float16
    in_maps = []
    for core in range(N_CORES):
        e = int(order[core])
        ce = counts[e]
        xpad = np.zeros((D_MODEL, C), cnp)
        xpad[:, :ce] = xf[tok_ids[e]].T.astype(cnp)
        w1e = np.ascontiguousarray(
            w1[e].reshape(DC, 128, FC, 128).transpose(2, 1, 0, 3)
            .reshape(FC, 128, D_MODEL).astype(cnp))
        b1e = np.ascontiguousarray(b1[e].reshape(FC, 128).T)
        b2e = np.ascontiguousarray(b2[e].reshape(DC, 128).T)
        im = {
            "w1p": w1e,
            "w2p": np.ascontiguousarray(w2[e].astype(cnp)),
            "b1p": b1e,
            "b2p": b2e,
        }
        for p, (t0, tn, _) in enumerate(passes):
            im[f"xp{p}"] = np.ascontiguousarray(
                xpad[:, t0:t0 + tn].reshape(DC, 128, tn))
        in_maps.append(im)

    if TRACE:
        _ensure_trace_hook()
    res = bass_utils.run_bass_kernel_spmd(
        nc, in_maps, core_ids=list(range(N_CORES)), trace=TRACE)
    LAST_EXEC_NS = res.exec_time_ns
    LAST_TRACE_PATH = (res.instructions_and_trace[1]
                       if res.instructions_and_trace else None)

    out = np.zeros((T, D_MODEL), np.float32)
    for core in range(N_CORES):
        e = int(order[core])
        ce = counts[e]
        ye = np.empty((D_MODEL, C), np.float32)
        for p, (t0, tn, _) in enumerate(passes):
            ye[:, t0:t0 + tn] = np.asarray(
                res.results[core][f"yp{p}"]).reshape(D_MODEL, tn)
        out[tok_ids[e]] += gates[e][:, None] * ye.T[:ce]

    return out.reshape(orig_shape)
